# revision 1
# baseline (speedup 1.0000x reference)
"""MHNA (masked, exp(n)-normalized multi-head attention) Trainium2 Bass kernel.

Sharding: 8 cores = batch(2) x head-groups(4 heads each). Each core computes its
4 heads' attention + its slice of the output projection (Wo rows); host sums the
4 partial outputs per batch and adds bo.

Device layout choices (all validated against a numpy mirror):
  - x is passed pre-transposed (xt = x[b].T) so every projection streams with the
    contraction dim (D) on partitions.
  - Q/K are produced transposed (QT/KT = W.T @ xT) in head-pair tiles [128, S]:
    partitions 0:64 = even head, 64:128 = odd head. Scores then run as K=64
    row-packed matmuls (two heads concurrently in the PE array).
  - The causal mask and the exp(n) normalizer: scores*mask/exp(n_t). The
    normalizer is folded into Q (q_t scaled by exp(-n_t) before scores); the
    mask is a single sliding-window tile maskB[128, 896] applied during the
    PSUM->SBUF evacuation of diagonal score blocks.
  - ctx is produced transposed (ctxT = V.T @ ST) with col-packed M=64 matmuls
    (two heads concurrently), which feeds the out-projection directly as lhsT.
  - Biases: per-partition biases (bq/bk/bn) via ACT/DVE ops; the V bias (varies
    along the free dim) via a K=1 rank-1 matmul accumulated into the same PSUM.
"""
import numpy as np

import concourse.bacc as bacc
import concourse.mybir as mybir
import concourse.tile as tile
from concourse.bass_utils import run_bass_kernel_spmd

F32 = mybir.dt.float32
MMDT = mybir.dt.float32r
AF = mybir.ActivationFunctionType
ALU = mybir.AluOpType

B, S, D, H, DH = 2, 2048, 1024, 16, 64
HL = 4            # heads per core
NTG = 4           # t groups of 512
NTC = 16          # t chunks of 128

_IN_SHAPES = dict(
    xt=(D, S), wq=(D, 256), wk=(D, 256), bq=(128, 2), bk=(128, 2),
    wv=(D, 256), bvr=(1, 256), wn=(D, HL), bnc=(HL, 1), wo=(256, D),
    sel=(HL, 256), maskB=(128, 896), ones=(1, 128),
)


def _kernel_body(tc, out, ins, phases=(1, 2, 3)):
    nc = tc.nc
    with (
        tc.tile_pool(name="const", bufs=1) as cp,
        tc.tile_pool(name="xtp", bufs=2) as xtp,
        tc.tile_pool(name="big", bufs=1) as bigp,
        tc.tile_pool(name="stp", bufs=6) as stp,
        tc.tile_pool(name="outp", bufs=2) as outp,
        tc.tile_pool(name="ps_st", bufs=3, space="PSUM") as ps_st,
        tc.tile_pool(name="ps_ctx", bufs=2, space="PSUM") as ps_ctx,
        tc.tile_pool(name="ps_gen", bufs=2, space="PSUM") as ps_gen,
        tc.tile_pool(name="ps_vn", bufs=1, space="PSUM") as ps_vn,
    ):
        # ---- constants / weights to SBUF ----
        wq_sb = cp.tile([128, 8, 256], MMDT)
        wk_sb = cp.tile([128, 8, 256], MMDT)
        wv_sb = cp.tile([128, 8, 256], MMDT)
        wn_sb = cp.tile([128, 8, HL], MMDT)
        wo_sb = cp.tile([128, 2, D], MMDT)
        nc.sync.dma_start(wq_sb[:], ins["wq"].rearrange("(a p) c -> p a c", p=128))
        nc.sync.dma_start(wk_sb[:], ins["wk"].rearrange("(a p) c -> p a c", p=128))
        nc.sync.dma_start(wv_sb[:], ins["wv"].rearrange("(a p) c -> p a c", p=128))
        nc.sync.dma_start(wn_sb[:], ins["wn"].rearrange("(a p) c -> p a c", p=128))
        nc.sync.dma_start(wo_sb[:], ins["wo"].rearrange("(a p) c -> p a c", p=128))
        bq_sb = cp.tile([128, 2], F32)
        bk_sb = cp.tile([128, 2], F32)
        bvr_sb = cp.tile([1, 256], MMDT)
        bnc_sb = cp.tile([HL, 1], F32)
        sel_sb = cp.tile([HL, 256], MMDT)
        mask_sb = cp.tile([128, 896], F32)
        ones_sb = cp.tile([1, 128], MMDT)
        for name, t in (("bq", bq_sb), ("bk", bk_sb), ("bvr", bvr_sb),
                        ("bnc", bnc_sb), ("sel", sel_sb), ("maskB", mask_sb),
                        ("ones", ones_sb)):
            nc.sync.dma_start(t[:], ins[name][:])

        qt_sb = bigp.tile([128, 2, S], MMDT)      # [part, pair, t]
        kt_sb = bigp.tile([128, 2, S], MMDT)
        v_sb = bigp.tile([128, NTC, 256], MMDT)   # [s-in-chunk, chunk, hc]
        wt_sb = bigp.tile([HL, S], MMDT)          # exp(-(n+bn)) per local head
        ctxt_sb = bigp.tile([128, 2, S], MMDT)    # [pair-dv, pair, t]

        xt_r = ins["xt"].rearrange("(a p) t -> p a t", p=128)

        # ================= stage 1: projections =================
        for tg in range(NTG if 1 in phases else 0):
            tsl = slice(tg * 512, (tg + 1) * 512)
            xt_tg = xtp.tile([128, 8, 512], MMDT, tag="xt")
            nc.sync.dma_start(xt_tg[:], xt_r[:, :, tsl])

            # N-projection -> wT = exp(-(n_pre + bn))
            n_ps = ps_vn.tile([HL, 512], F32, tag="v")
            for dc in range(8):
                nc.tensor.matmul(n_ps[:], wn_sb[:, dc, :], xt_tg[:, dc, :],
                                 start=(dc == 0), stop=(dc == 7))
            nc.scalar.activation(wt_sb[:, tsl], n_ps[:], AF.Exp,
                                 bias=bnc_sb[:], scale=-1.0)

            for pair in range(2):
                psl = slice(128 * pair, 128 * pair + 128)
                # wrep[p, t] = exp(-n) broadcast: partitions 0:64 <- even head
                wrep_ps = ps_gen.tile([128, 512], F32, tag="gen")
                nc.tensor.matmul(wrep_ps[:], sel_sb[:, psl], wt_sb[:, tsl],
                                 start=True, stop=True)
                wrep_sb = outp.tile([128, 512], F32, tag="wrep_sb")
                nc.scalar.copy(wrep_sb[:], wrep_ps[:])
                # QT
                q_ps = ps_gen.tile([128, 512], F32, tag="gen")
                for dc in range(8):
                    nc.tensor.matmul(q_ps[:], wq_sb[:, dc, psl], xt_tg[:, dc, :],
                                     start=(dc == 0), stop=(dc == 7))
                nc.vector.scalar_tensor_tensor(
                    qt_sb[:, pair, tsl], q_ps[:], bq_sb[:, pair:pair + 1],
                    wrep_sb[:], ALU.add, ALU.mult)
                # KT
                k_ps = ps_gen.tile([128, 512], F32, tag="gen")
                for dc in range(8):
                    nc.tensor.matmul(k_ps[:], wk_sb[:, dc, psl], xt_tg[:, dc, :],
                                     start=(dc == 0), stop=(dc == 7))
                nc.scalar.activation(kt_sb[:, pair, tsl], k_ps[:], AF.Identity,
                                     bias=bk_sb[:, pair:pair + 1])

            # V (+bias via rank-1 matmul)
            for tl in range(4):
                tc16 = tg * 4 + tl
                v_ps = ps_vn.tile([128, 256], F32, tag="v")
                for dc in range(8):
                    nc.tensor.matmul(v_ps[:], xt_tg[:, dc, tl * 128:(tl + 1) * 128],
                                     wv_sb[:, dc, :], start=(dc == 0), stop=False)
                nc.tensor.matmul(v_ps[:], ones_sb[:], bvr_sb[:],
                                 start=False, stop=True)
                if tl % 2 == 0:
                    nc.vector.tensor_copy(v_sb[:, tc16, :], v_ps[:])
                else:
                    nc.scalar.copy(v_sb[:, tc16, :], v_ps[:])

        # ================= stage 2+3: scores + ctx =================
        ndve = 0
        for pair in range(2 if 2 in phases else 0):
            for tg in range(NTG):
                tsl = slice(tg * 512, (tg + 1) * 512)
                ctx_ps = [ps_ctx.tile([64, 512], F32, tag="ctx", name=f"ctx{_h}") for _h in range(2)]
                nblk = 4 * tg + 4
                prev_sb, prev_j = None, -1
                for j in range(nblk):
                    st_list = []
                    for hh in range(2):
                        hsl = slice(64 * hh, 64 * hh + 64)
                        st_ps = ps_st.tile([128, 512], F32, tag="st")
                        nc.tensor.matmul(
                            st_ps[:], kt_sb[hsl, pair, j * 128:(j + 1) * 128],
                            qt_sb[hsl, pair, tsl], start=True, stop=True,
                            tile_position=(64 * hh, 0))
                        st_list.append(st_ps)
                    cur_sb = []
                    for hh in range(2):
                        st_sb = stp.tile([128, 512], MMDT, tag="st_sb")
                        r = j - 4 * tg
                        if r >= 0:
                            nc.vector.tensor_mul(
                                st_sb[:], st_list[hh][:],
                                mask_sb[:, 384 - 128 * r: 896 - 128 * r])
                        else:
                            ndve += 1
                            if ndve % 4 == 0:
                                nc.vector.tensor_copy(st_sb[:], st_list[hh][:])
                            else:
                                nc.scalar.copy(st_sb[:], st_list[hh][:])
                        cur_sb.append(st_sb)
                    if prev_sb is not None:
                        for hh in range(2):
                            hl_g = 2 * pair + hh
                            nc.tensor.matmul(
                                ctx_ps[hh][:],
                                v_sb[:, prev_j, 64 * hl_g:64 * hl_g + 64],
                                prev_sb[hh][:],
                                start=(prev_j == 0), stop=False)
                    prev_sb, prev_j = cur_sb, j
                for hh in range(2):
                    hl_g = 2 * pair + hh
                    nc.tensor.matmul(
                        ctx_ps[hh][:],
                        v_sb[:, prev_j, 64 * hl_g:64 * hl_g + 64], prev_sb[hh][:],
                        start=(prev_j == 0), stop=True)
                for hh in range(2):
                    if (tg + hh) % 2 == 0:
                        nc.vector.tensor_copy(ctxt_sb[64*hh:64*hh+64, pair, tsl], ctx_ps[hh][:])
                    else:
                        nc.scalar.copy(ctxt_sb[64*hh:64*hh+64, pair, tsl], ctx_ps[hh][:])

        # ================= stage 4: out projection =================
        for tc16 in range(NTC if 3 in phases else 0):
            csl = slice(tc16 * 128, (tc16 + 1) * 128)
            out_sb = outp.tile([128, D], F32, tag="out")
            for eb in range(2):
                esl = slice(eb * 512, (eb + 1) * 512)
                o_ps = ps_gen.tile([128, 512], F32, tag="gen")
                for pair in range(2):
                    nc.tensor.matmul(o_ps[:], ctxt_sb[:, pair, csl],
                                     wo_sb[:, pair, esl],
                                     start=(pair == 0), stop=(pair == 1))
                if eb == 0:
                    nc.vector.tensor_copy(out_sb[:, esl], o_ps[:])
                else:
                    nc.scalar.copy(out_sb[:, esl], o_ps[:])
            nc.sync.dma_start(out[csl, :], out_sb[:])


def build_nc(phases=(1, 2, 3)):
    nc = bacc.Bacc("TRN2", target_bir_lowering=False, debug=False, num_devices=8)
    _mm = {"xt", "wq", "wk", "wv", "wn", "wo", "sel", "ones", "bvr"}
    ins = {k: nc.dram_tensor(k, list(s), MMDT if k in _mm else F32,
                             kind="ExternalInput").ap()
           for k, s in _IN_SHAPES.items()}
    out = nc.dram_tensor("out", [S, D], F32, kind="ExternalOutput").ap()
    with tile.TileContext(nc) as tc:
        _kernel_body(tc, out, ins, phases=phases)
    nc.compile()
    return nc


def _make_maskB():
    m = np.zeros((128, 896), dtype=np.float32)
    s = np.arange(128)[:, None]
    c = np.arange(896)[None, :]
    m[(c >= 384) & ((c - 384) >= s)] = 1.0
    m[:, 512:] = 1.0
    return m


def core_inputs(inp, c):
    b, hg = c // 4, c % 4
    heads = list(range(4 * hg, 4 * hg + 4))
    x = np.asarray(inp["x"], dtype=np.float32)
    Wqk = np.asarray(inp["Wqk"], dtype=np.float32)
    bqk = np.asarray(inp["bqk"], dtype=np.float32)
    Wv = np.asarray(inp["Wv"], dtype=np.float32)
    bv = np.asarray(inp["bv"], dtype=np.float32)
    Wn = np.asarray(inp["Wn"], dtype=np.float32)
    bn = np.asarray(inp["bn"], dtype=np.float32)
    Wo = np.asarray(inp["Wo"], dtype=np.float32)
    d = {}
    d["xt"] = x[b].T
    d["wq"] = np.concatenate([Wqk[:, h * 64:(h + 1) * 64] for h in heads], 1)
    d["wk"] = np.concatenate([Wqk[:, 1024 + h * 64:1024 + (h + 1) * 64] for h in heads], 1)
    d["bq"] = np.concatenate([bqk[h * 64:(h + 1) * 64] for h in heads]).reshape(2, 128).T
    d["bk"] = np.concatenate([bqk[1024 + h * 64:1024 + (h + 1) * 64] for h in heads]).reshape(2, 128).T
    d["wv"] = np.concatenate([Wv[:, h * 64:(h + 1) * 64] for h in heads], 1)
    d["bvr"] = np.concatenate([bv[h * 64:(h + 1) * 64] for h in heads]).reshape(1, 256)
    d["wn"] = Wn[:, heads]
    d["bnc"] = -bn[heads].reshape(4, 1)
    d["wo"] = np.concatenate([Wo[h * 64:(h + 1) * 64, :] for h in heads], 0)
    sel = np.zeros((4, 256), dtype=np.float32)
    for p in range(2):
        sel[2 * p + 0, 128 * p:128 * p + 64] = 1.0
        sel[2 * p + 1, 128 * p + 64:128 * p + 128] = 1.0
    d["sel"] = sel
    d["maskB"] = _make_maskB()
    d["ones"] = np.ones((1, 128), dtype=np.float32)
    return {k: np.ascontiguousarray(v, dtype=np.float32) for k, v in d.items()}


_NC_CACHE = {}


def _get_nc():
    if "nc" not in _NC_CACHE:
        _NC_CACHE["nc"] = build_nc()
    return _NC_CACHE["nc"]


def _run(inputs, **spmd_kwargs):
    nc = _get_nc()
    in_maps = [core_inputs(inputs, c) for c in range(8)]
    res = run_bass_kernel_spmd(nc, in_maps, list(range(8)), **spmd_kwargs)
    bo = np.asarray(inputs["bo"], dtype=np.float32)
    out = np.stack([
        res.results[0 + 4 * b]["out"] + res.results[1 + 4 * b]["out"]
        + res.results[2 + 4 * b]["out"] + res.results[3 + 4 * b]["out"] + bo
        for b in range(B)
    ])
    return out.astype(np.float32), res


def kernel(**inputs):
    out, _ = _run(inputs)
    return out



# revision 2
# speedup vs baseline: 7.7222x; 7.7222x over previous
"""MHNA (masked, exp(n)-normalized multi-head attention) Trainium2 Bass kernel.

Sharding: 8 cores = batch(2) x head-groups(4 heads each). Each core computes its
4 heads' attention + its slice of the output projection (Wo rows); host sums the
4 partial outputs per batch and adds bo.

I/O design (the dominant cost through this harness is per-call host<->device
I/O staging, not device compute): ALL inputs are packed into a single bf16
blob per core (one buffer instead of 17), and the partial output is written
in bf16. Device compute accumulates in fp32 PSUM throughout; only SBUF
residents are bf16.

Device layout (same structure as the validated fp32r version):
  - x is passed pre-transposed (xt = x[b].T) so every projection streams with
    the contraction dim (D) on partitions.
  - Q/K are produced transposed (QT/KT = W.T @ xT) in head-pair tiles [128, S]:
    partitions 0:64 = even head, 64:128 = odd head. Scores then run as K=64
    row-packed matmuls (two heads concurrently in the PE array).
  - scores*mask/exp(n_t): the normalizer is folded into Q (q_t scaled by
    exp(-n_t) before scores); the causal mask is a sliding-window tile
    maskB[128, 896] applied during PSUM->SBUF evacuation of diagonal blocks.
  - ctx is produced transposed (ctxT = V.T @ ST) with col-packed M=64 matmuls,
    feeding the out-projection directly as lhsT.
  - Biases: per-partition biases (bq/bk/bn) via ACT/DVE ops (converted to fp32
    tiles on device); the V bias (varies along the free dim) via a K=1 rank-1
    matmul accumulated into the same PSUM.
"""
import numpy as np

import concourse.bacc as bacc
import concourse.mybir as mybir
import concourse.tile as tile
from concourse.bass_utils import run_bass_kernel_spmd

F32 = mybir.dt.float32
BF16 = mybir.dt.bfloat16
NPBF = mybir.dt.np(mybir.dt.bfloat16)
AF = mybir.ActivationFunctionType
ALU = mybir.AluOpType

B, S, D, H, DH = 2, 2048, 1024, 16, 64
HL = 4            # heads per core
NTG = 4           # t groups of 512
NTC = 16          # t chunks of 128

# Packed-blob layout: (name, n_elems). Offsets accumulate in order; each
# section is 64-element aligned. Element order per section is chosen so the
# SBUF-load DMA sees large contiguous per-partition chunks:
#   xt   (a p t)  : xt[d=a*128+p, t] natural order, d-major
#   wq/wk/wv (p a c), wo (p a e), wn (p a c): per-partition contiguous
#   maskB/bq/bk (p c), sel (h c), bvr/ones (1 c), bnc (h 1)
_SECTS = [
    ("xt", D * S), ("wq", D * 256), ("wk", D * 256), ("wv", D * 256),
    ("wo", 256 * D), ("wn", D * HL), ("maskB", 128 * 896), ("sel", HL * 256),
    ("bq", 128 * 2), ("bk", 128 * 2), ("bvr", 256), ("ones", 128), ("bnc", 64),
]
_OFS = {}
_cur = 0
for _n, _sz in _SECTS:
    _OFS[_n] = _cur
    _cur += (_sz + 63) // 64 * 64
BLOB_N = _cur


def _views(blob):
    o = _OFS
    def sl(name, n):
        return blob[o[name]:o[name] + n]
    return dict(
        xt=sl("xt", D * S).rearrange("(a p t) -> p a t", p=128, t=S),
        wq=sl("wq", D * 256).rearrange("(p a c) -> p a c", a=8, c=256),
        wk=sl("wk", D * 256).rearrange("(p a c) -> p a c", a=8, c=256),
        wv=sl("wv", D * 256).rearrange("(p a c) -> p a c", a=8, c=256),
        wo=sl("wo", 256 * D).rearrange("(p a e) -> p a e", a=2, e=D),
        wn=sl("wn", D * HL).rearrange("(p a c) -> p a c", a=8, c=HL),
        maskB=sl("maskB", 128 * 896).rearrange("(p c) -> p c", c=896),
        sel=sl("sel", HL * 256).rearrange("(h c) -> h c", c=256),
        bq=sl("bq", 128 * 2).rearrange("(p c) -> p c", c=2),
        bk=sl("bk", 128 * 2).rearrange("(p c) -> p c", c=2),
        bvr=sl("bvr", 256).rearrange("(o c) -> o c", o=1),
        ones=sl("ones", 128).rearrange("(o c) -> o c", o=1),
        bnc=sl("bnc", HL).rearrange("(h o) -> h o", o=1),
    )


def _kernel_body(tc, out, blob, phases=(1, 2, 3)):
    nc = tc.nc
    ins = _views(blob)
    with (
        tc.tile_pool(name="const", bufs=1) as cp,
        tc.tile_pool(name="xtp", bufs=2) as xtp,
        tc.tile_pool(name="big", bufs=1) as bigp,
        tc.tile_pool(name="stp", bufs=6) as stp,
        tc.tile_pool(name="outp", bufs=2) as outp,
        tc.tile_pool(name="ps_st", bufs=3, space="PSUM") as ps_st,
        tc.tile_pool(name="ps_ctx", bufs=2, space="PSUM") as ps_ctx,
        tc.tile_pool(name="ps_gen", bufs=2, space="PSUM") as ps_gen,
        tc.tile_pool(name="ps_vn", bufs=1, space="PSUM") as ps_vn,
    ):
        # ---- constants / weights to SBUF ----
        wq_sb = cp.tile([128, 8, 256], BF16)
        wk_sb = cp.tile([128, 8, 256], BF16)
        wv_sb = cp.tile([128, 8, 256], BF16)
        wn_sb = cp.tile([128, 8, HL], BF16)
        wo_sb = cp.tile([128, 2, D], BF16)
        for name, t in (("wq", wq_sb), ("wk", wk_sb), ("wv", wv_sb),
                        ("wn", wn_sb), ("wo", wo_sb)):
            nc.sync.dma_start(t[:], ins[name])
        bq_bf = cp.tile([128, 2], BF16)
        bk_bf = cp.tile([128, 2], BF16)
        bnc_bf = cp.tile([HL, 1], BF16)
        bvr_sb = cp.tile([1, 256], BF16)
        sel_sb = cp.tile([HL, 256], BF16)
        mask_sb = cp.tile([128, 896], BF16)
        ones_sb = cp.tile([1, 128], BF16)
        for name, t in (("bq", bq_bf), ("bk", bk_bf), ("bvr", bvr_sb),
                        ("bnc", bnc_bf), ("sel", sel_sb), ("maskB", mask_sb),
                        ("ones", ones_sb)):
            nc.sync.dma_start(t[:], ins[name])
        # per-partition bias operands need fp32
        bq_sb = cp.tile([128, 2], F32)
        bk_sb = cp.tile([128, 2], F32)
        bnc_sb = cp.tile([HL, 1], F32)
        nc.scalar.copy(bq_sb[:], bq_bf[:])
        nc.scalar.copy(bk_sb[:], bk_bf[:])
        nc.scalar.copy(bnc_sb[:], bnc_bf[:])

        qt_sb = bigp.tile([128, 2, S], BF16)      # [part, pair, t]
        kt_sb = bigp.tile([128, 2, S], BF16)
        v_sb = bigp.tile([128, NTC, 256], BF16)   # [s-in-chunk, chunk, hc]
        wt_sb = bigp.tile([HL, S], BF16)          # exp(-(n+bn)) per local head
        ctxt_sb = bigp.tile([128, 2, S], BF16)    # [pair-dv, pair, t]

        # ================= stage 1: projections =================
        for tg in range(NTG if 1 in phases else 0):
            tsl = slice(tg * 512, (tg + 1) * 512)
            xt_tg = xtp.tile([128, 8, 512], BF16, tag="xt")
            nc.sync.dma_start(xt_tg[:], ins["xt"][:, :, tsl])

            # N-projection -> wT = exp(-(n_pre + bn))
            n_ps = ps_vn.tile([HL, 512], F32, tag="v")
            for dc in range(8):
                nc.tensor.matmul(n_ps[:], wn_sb[:, dc, :], xt_tg[:, dc, :],
                                 start=(dc == 0), stop=(dc == 7))
            nc.scalar.activation(wt_sb[:, tsl], n_ps[:], AF.Exp,
                                 bias=bnc_sb[:], scale=-1.0)

            for pair in range(2):
                psl = slice(128 * pair, 128 * pair + 128)
                # wrep[p, t] = exp(-n) broadcast: partitions 0:64 <- even head
                wrep_ps = ps_gen.tile([128, 512], F32, tag="gen")
                nc.tensor.matmul(wrep_ps[:], sel_sb[:, psl], wt_sb[:, tsl],
                                 start=True, stop=True)
                wrep_sb = outp.tile([128, 512], F32, tag="wrep_sb")
                nc.scalar.copy(wrep_sb[:], wrep_ps[:])
                # QT
                q_ps = ps_gen.tile([128, 512], F32, tag="gen")
                for dc in range(8):
                    nc.tensor.matmul(q_ps[:], wq_sb[:, dc, psl], xt_tg[:, dc, :],
                                     start=(dc == 0), stop=(dc == 7))
                nc.vector.scalar_tensor_tensor(
                    qt_sb[:, pair, tsl], q_ps[:], bq_sb[:, pair:pair + 1],
                    wrep_sb[:], ALU.add, ALU.mult)
                # KT
                k_ps = ps_gen.tile([128, 512], F32, tag="gen")
                for dc in range(8):
                    nc.tensor.matmul(k_ps[:], wk_sb[:, dc, psl], xt_tg[:, dc, :],
                                     start=(dc == 0), stop=(dc == 7))
                nc.scalar.activation(kt_sb[:, pair, tsl], k_ps[:], AF.Identity,
                                     bias=bk_sb[:, pair:pair + 1])

            # V (+bias via rank-1 matmul)
            for tl in range(4):
                tc16 = tg * 4 + tl
                v_ps = ps_vn.tile([128, 256], F32, tag="v")
                for dc in range(8):
                    nc.tensor.matmul(v_ps[:], xt_tg[:, dc, tl * 128:(tl + 1) * 128],
                                     wv_sb[:, dc, :], start=(dc == 0), stop=False)
                nc.tensor.matmul(v_ps[:], ones_sb[:], bvr_sb[:],
                                 start=False, stop=True)
                if tl % 2 == 0:
                    nc.vector.tensor_copy(v_sb[:, tc16, :], v_ps[:])
                else:
                    nc.scalar.copy(v_sb[:, tc16, :], v_ps[:])

        # ================= stage 2+3: scores + ctx =================
        ndve = 0
        for pair in range(2 if 2 in phases else 0):
            for tg in range(NTG):
                tsl = slice(tg * 512, (tg + 1) * 512)
                ctx_ps = [ps_ctx.tile([64, 512], F32, tag="ctx", name=f"ctx{_h}") for _h in range(2)]
                nblk = 4 * tg + 4
                prev_sb, prev_j = None, -1
                for j in range(nblk):
                    st_list = []
                    for hh in range(2):
                        hsl = slice(64 * hh, 64 * hh + 64)
                        st_ps = ps_st.tile([128, 512], F32, tag="st")
                        nc.tensor.matmul(
                            st_ps[:], kt_sb[hsl, pair, j * 128:(j + 1) * 128],
                            qt_sb[hsl, pair, tsl], start=True, stop=True,
                            tile_position=(64 * hh, 0))
                        st_list.append(st_ps)
                    cur_sb = []
                    for hh in range(2):
                        st_sb = stp.tile([128, 512], BF16, tag="st_sb")
                        r = j - 4 * tg
                        if r >= 0:
                            nc.vector.tensor_mul(
                                st_sb[:], st_list[hh][:],
                                mask_sb[:, 384 - 128 * r: 896 - 128 * r])
                        else:
                            ndve += 1
                            if ndve % 4 == 0:
                                nc.vector.tensor_copy(st_sb[:], st_list[hh][:])
                            else:
                                nc.scalar.copy(st_sb[:], st_list[hh][:])
                        cur_sb.append(st_sb)
                    if prev_sb is not None:
                        for hh in range(2):
                            hl_g = 2 * pair + hh
                            nc.tensor.matmul(
                                ctx_ps[hh][:],
                                v_sb[:, prev_j, 64 * hl_g:64 * hl_g + 64],
                                prev_sb[hh][:],
                                start=(prev_j == 0), stop=False)
                    prev_sb, prev_j = cur_sb, j
                for hh in range(2):
                    hl_g = 2 * pair + hh
                    nc.tensor.matmul(
                        ctx_ps[hh][:],
                        v_sb[:, prev_j, 64 * hl_g:64 * hl_g + 64], prev_sb[hh][:],
                        start=(prev_j == 0), stop=True)
                for hh in range(2):
                    if (tg + hh) % 2 == 0:
                        nc.vector.tensor_copy(ctxt_sb[64*hh:64*hh+64, pair, tsl], ctx_ps[hh][:])
                    else:
                        nc.scalar.copy(ctxt_sb[64*hh:64*hh+64, pair, tsl], ctx_ps[hh][:])

        # ================= stage 4: out projection =================
        for tc16 in range(NTC if 3 in phases else 0):
            csl = slice(tc16 * 128, (tc16 + 1) * 128)
            out_sb = outp.tile([128, D], BF16, tag="out")
            for eb in range(2):
                esl = slice(eb * 512, (eb + 1) * 512)
                o_ps = ps_gen.tile([128, 512], F32, tag="gen")
                for pair in range(2):
                    nc.tensor.matmul(o_ps[:], ctxt_sb[:, pair, csl],
                                     wo_sb[:, pair, esl],
                                     start=(pair == 0), stop=(pair == 1))
                if eb == 0:
                    nc.vector.tensor_copy(out_sb[:, esl], o_ps[:])
                else:
                    nc.scalar.copy(out_sb[:, esl], o_ps[:])
            nc.sync.dma_start(out[csl, :], out_sb[:])


def build_nc(phases=(1, 2, 3)):
    nc = bacc.Bacc("TRN2", target_bir_lowering=False, debug=False,
                   enable_partition_id=False)
    blob = nc.dram_tensor("blob", [BLOB_N], BF16, kind="ExternalInput").ap()
    out = nc.dram_tensor("out", [S, D], BF16, kind="ExternalOutput").ap()
    with tile.TileContext(nc) as tc:
        _kernel_body(tc, out, blob, phases=phases)
    nc.compile()
    return nc


def _make_maskB():
    m = np.zeros((128, 896), dtype=np.float32)
    s = np.arange(128)[:, None]
    c = np.arange(896)[None, :]
    m[(c >= 384) & ((c - 384) >= s)] = 1.0
    m[:, 512:] = 1.0
    return m


def core_inputs(inp, c):
    b, hg = c // 4, c % 4
    heads = list(range(4 * hg, 4 * hg + 4))
    x = np.asarray(inp["x"], dtype=np.float32)
    Wqk = np.asarray(inp["Wqk"], dtype=np.float32)
    bqk = np.asarray(inp["bqk"], dtype=np.float32)
    Wv = np.asarray(inp["Wv"], dtype=np.float32)
    bv = np.asarray(inp["bv"], dtype=np.float32)
    Wn = np.asarray(inp["Wn"], dtype=np.float32)
    bn = np.asarray(inp["bn"], dtype=np.float32)
    Wo = np.asarray(inp["Wo"], dtype=np.float32)

    def perm_pac(w, a, c_):
        # [128*a, c_] -> flat in (p, a, c) order
        return np.ascontiguousarray(
            w.reshape(a, 128, c_).transpose(1, 0, 2)).reshape(-1)

    d = {}
    d["xt"] = x[b].T.reshape(-1)                       # (a p t) == d-major
    wq = np.concatenate([Wqk[:, h * 64:(h + 1) * 64] for h in heads], 1)
    wk = np.concatenate([Wqk[:, 1024 + h * 64:1024 + (h + 1) * 64] for h in heads], 1)
    wv = np.concatenate([Wv[:, h * 64:(h + 1) * 64] for h in heads], 1)
    wo = np.concatenate([Wo[h * 64:(h + 1) * 64, :] for h in heads], 0)
    d["wq"] = perm_pac(wq, 8, 256)
    d["wk"] = perm_pac(wk, 8, 256)
    d["wv"] = perm_pac(wv, 8, 256)
    d["wo"] = perm_pac(wo, 2, 1024)
    d["wn"] = perm_pac(Wn[:, heads], 8, HL)
    d["maskB"] = _make_maskB().reshape(-1)
    sel = np.zeros((4, 256), dtype=np.float32)
    for p in range(2):
        sel[2 * p + 0, 128 * p:128 * p + 64] = 1.0
        sel[2 * p + 1, 128 * p + 64:128 * p + 128] = 1.0
    d["sel"] = sel.reshape(-1)
    d["bq"] = np.concatenate([bqk[h * 64:(h + 1) * 64] for h in heads]).reshape(2, 128).T.reshape(-1)
    d["bk"] = np.concatenate([bqk[1024 + h * 64:1024 + (h + 1) * 64] for h in heads]).reshape(2, 128).T.reshape(-1)
    d["bvr"] = np.concatenate([bv[h * 64:(h + 1) * 64] for h in heads])
    d["ones"] = np.ones(128, dtype=np.float32)
    bnc = np.zeros(64, dtype=np.float32)
    bnc[:HL] = -bn[heads]
    d["bnc"] = bnc

    blob = np.zeros(BLOB_N, dtype=NPBF)
    for name, sz in _SECTS:
        v = d[name].astype(np.float32).reshape(-1)
        blob[_OFS[name]:_OFS[name] + v.size] = v.astype(NPBF)
    return {"blob": blob}


_NC_CACHE = {}


def _get_nc():
    if "nc" not in _NC_CACHE:
        _NC_CACHE["nc"] = build_nc()
    return _NC_CACHE["nc"]


def _run(inputs, **spmd_kwargs):
    nc = _get_nc()
    in_maps = [core_inputs(inputs, c) for c in range(8)]
    res = run_bass_kernel_spmd(nc, in_maps, list(range(8)), **spmd_kwargs)
    bo = np.asarray(inputs["bo"], dtype=np.float32)
    out = np.stack([
        res.results[0 + 4 * b]["out"].astype(np.float32)
        + res.results[1 + 4 * b]["out"].astype(np.float32)
        + res.results[2 + 4 * b]["out"].astype(np.float32)
        + res.results[3 + 4 * b]["out"].astype(np.float32) + bo
        for b in range(B)
    ])
    return out.astype(np.float32), res


def kernel(**inputs):
    out, _ = _run(inputs)
    return out


# revision 4
# speedup vs baseline: 61.0574x; 7.9067x over previous
"""MHNA (masked, exp(n)-normalized multi-head attention) Trainium2 Bass kernel.

Sharding: 8 cores = batch(2) x head-groups(4 heads each). Each core computes its
4 heads' attention + its slice of the output projection (Wo rows); host sums the
4 partial outputs per batch and adds bo.

I/O design (the dominant cost through this harness is per-call host<->device
I/O staging, not device compute): ALL inputs are packed into a single bf16
blob per core (one buffer instead of 17), and the partial output is written
in bf16. Device compute accumulates in fp32 PSUM throughout; only SBUF
residents are bf16.

Device layout (same structure as the validated fp32r version):
  - x is passed pre-transposed (xt = x[b].T) so every projection streams with
    the contraction dim (D) on partitions.
  - Q/K are produced transposed (QT/KT = W.T @ xT) in head-pair tiles [128, S]:
    partitions 0:64 = even head, 64:128 = odd head. Scores then run as K=64
    row-packed matmuls (two heads concurrently in the PE array).
  - scores*mask/exp(n_t): the normalizer is folded into Q (q_t scaled by
    exp(-n_t) before scores); the causal mask is a sliding-window tile
    maskB[128, 896] applied during PSUM->SBUF evacuation of diagonal blocks.
  - ctx is produced transposed (ctxT = V.T @ ST) with col-packed M=64 matmuls,
    feeding the out-projection directly as lhsT.
  - Biases: per-partition biases (bq/bk/bn) via ACT/DVE ops (converted to fp32
    tiles on device); the V bias (varies along the free dim) via a K=1 rank-1
    matmul accumulated into the same PSUM.
"""
import numpy as np

import concourse.bacc as bacc
import concourse.mybir as mybir
import concourse.tile as tile
from concourse.bass_utils import run_bass_kernel_spmd

F32 = mybir.dt.float32
BF16 = mybir.dt.bfloat16
NPBF = mybir.dt.np(mybir.dt.bfloat16)
AF = mybir.ActivationFunctionType
ALU = mybir.AluOpType

B, S, D, H, DH = 2, 2048, 1024, 16, 64
HL = 4            # heads per core
NTG = 4           # t groups of 512
NTC = 16          # t chunks of 128

# Packed-blob layout: (name, n_elems). Offsets accumulate in order; each
# section is 64-element aligned. Element order per section is chosen so the
# SBUF-load DMA sees large contiguous per-partition chunks:
#   xt   (a p t)  : xt[d=a*128+p, t] natural order, d-major
#   wq/wk/wv (p a c), wo (p a e), wn (p a c): per-partition contiguous
#   maskB/bq/bk (p c), sel (h c), bvr/ones (1 c), bnc (h 1)
_SECTS = [
    ("xt", D * S), ("wq", D * 256), ("wk", D * 256), ("wv", D * 256),
    ("wo", 256 * D), ("wn", D * HL), ("maskB", 128 * 896), ("sel", HL * 256),
    ("bq", 128 * 2), ("bk", 128 * 2), ("bvr", 256), ("ones", 128), ("bnc", 64),
]
_OFS = {}
_cur = 0
for _n, _sz in _SECTS:
    _OFS[_n] = _cur
    _cur += (_sz + 63) // 64 * 64
BLOB_N = _cur


def _views(blob):
    o = _OFS
    def sl(name, n):
        return blob[o[name]:o[name] + n]
    return dict(
        xt=sl("xt", D * S).rearrange("(a p t) -> p a t", p=128, t=S),
        wq=sl("wq", D * 256).rearrange("(p a c) -> p a c", a=8, c=256),
        wk=sl("wk", D * 256).rearrange("(p a c) -> p a c", a=8, c=256),
        wv=sl("wv", D * 256).rearrange("(p a c) -> p a c", a=8, c=256),
        wo=sl("wo", 256 * D).rearrange("(p a e) -> p a e", a=2, e=D),
        wn=sl("wn", D * HL).rearrange("(p a c) -> p a c", a=8, c=HL),
        maskB=sl("maskB", 128 * 896).rearrange("(p c) -> p c", c=896),
        sel=sl("sel", HL * 256).rearrange("(h c) -> h c", c=256),
        bq=sl("bq", 128 * 2).rearrange("(p c) -> p c", c=2),
        bk=sl("bk", 128 * 2).rearrange("(p c) -> p c", c=2),
        bvr=sl("bvr", 256).rearrange("(o c) -> o c", o=1),
        ones=sl("ones", 128).rearrange("(o c) -> o c", o=1),
        bnc=sl("bnc", HL).rearrange("(h o) -> h o", o=1),
    )


def _kernel_body(tc, out, blob, phases=(1, 2, 3), reps=1):
    nc = tc.nc
    ins = _views(blob)
    with (
        tc.tile_pool(name="const", bufs=1) as cp,
        tc.tile_pool(name="xtp", bufs=2) as xtp,
        tc.tile_pool(name="big", bufs=2) as bigp,
        tc.tile_pool(name="stp", bufs=6) as stp,
        tc.tile_pool(name="outp", bufs=2) as outp,
        tc.tile_pool(name="ps_st", bufs=3, space="PSUM") as ps_st,
        tc.tile_pool(name="ps_ctx", bufs=2, space="PSUM") as ps_ctx,
        tc.tile_pool(name="ps_gen", bufs=2, space="PSUM") as ps_gen,
        tc.tile_pool(name="ps_vn", bufs=1, space="PSUM") as ps_vn,
    ):
        for _rep in range(reps):
            _one_pass(nc, out, ins, phases, cp, xtp, bigp, stp, outp,
                      ps_st, ps_ctx, ps_gen, ps_vn)


def _one_pass(nc, out, ins, phases, cp, xtp, bigp, stp, outp,
              ps_st, ps_ctx, ps_gen, ps_vn):
        # ---- constants / weights to SBUF ----
        wq_sb = cp.tile([128, 8, 256], BF16)
        wk_sb = cp.tile([128, 8, 256], BF16)
        wv_sb = cp.tile([128, 8, 256], BF16)
        wn_sb = cp.tile([128, 8, HL], BF16)
        wo_sb = cp.tile([128, 2, D], BF16)
        for name, t in (("wq", wq_sb), ("wk", wk_sb), ("wv", wv_sb),
                        ("wn", wn_sb), ("wo", wo_sb)):
            nc.sync.dma_start(t[:], ins[name])
        bq_bf = cp.tile([128, 2], BF16)
        bk_bf = cp.tile([128, 2], BF16)
        bnc_bf = cp.tile([HL, 1], BF16)
        bvr_sb = cp.tile([1, 256], BF16)
        sel_sb = cp.tile([HL, 256], BF16)
        mask_sb = cp.tile([128, 896], BF16)
        ones_sb = cp.tile([1, 128], BF16)
        for name, t in (("bq", bq_bf), ("bk", bk_bf), ("bvr", bvr_sb),
                        ("bnc", bnc_bf), ("sel", sel_sb), ("maskB", mask_sb),
                        ("ones", ones_sb)):
            nc.sync.dma_start(t[:], ins[name])
        # per-partition bias operands need fp32
        bq_sb = cp.tile([128, 2], F32)
        bk_sb = cp.tile([128, 2], F32)
        bnc_sb = cp.tile([HL, 1], F32)
        nc.scalar.copy(bq_sb[:], bq_bf[:])
        nc.scalar.copy(bk_sb[:], bk_bf[:])
        nc.scalar.copy(bnc_sb[:], bnc_bf[:])

        qt_sb = bigp.tile([128, 2, S], BF16)      # [part, pair, t]
        kt_sb = bigp.tile([128, 2, S], BF16)
        v_sb = bigp.tile([128, NTC, 256], BF16)   # [s-in-chunk, chunk, hc]
        wt_sb = bigp.tile([HL, S], BF16)          # exp(-(n+bn)) per local head
        ctxt_sb = bigp.tile([128, 2, S], BF16)    # [pair-dv, pair, t]

        # ================= stage 1: projections =================
        for tg in range(NTG if 1 in phases else 0):
            tsl = slice(tg * 512, (tg + 1) * 512)
            xt_tg = xtp.tile([128, 8, 512], BF16, tag="xt")
            nc.sync.dma_start(xt_tg[:], ins["xt"][:, :, tsl])

            # N-projection -> wT = exp(-(n_pre + bn))
            n_ps = ps_vn.tile([HL, 512], F32, tag="v")
            for dc in range(8):
                nc.tensor.matmul(n_ps[:], wn_sb[:, dc, :], xt_tg[:, dc, :],
                                 start=(dc == 0), stop=(dc == 7))
            nc.scalar.activation(wt_sb[:, tsl], n_ps[:], AF.Exp,
                                 bias=bnc_sb[:], scale=-1.0)

            for pair in range(2):
                psl = slice(128 * pair, 128 * pair + 128)
                # wrep[p, t] = exp(-n) broadcast: partitions 0:64 <- even head
                wrep_ps = ps_gen.tile([128, 512], F32, tag="gen")
                nc.tensor.matmul(wrep_ps[:], sel_sb[:, psl], wt_sb[:, tsl],
                                 start=True, stop=True)
                wrep_sb = outp.tile([128, 512], F32, tag="wrep_sb")
                nc.scalar.copy(wrep_sb[:], wrep_ps[:])
                # QT
                q_ps = ps_gen.tile([128, 512], F32, tag="gen")
                for dc in range(8):
                    nc.tensor.matmul(q_ps[:], wq_sb[:, dc, psl], xt_tg[:, dc, :],
                                     start=(dc == 0), stop=(dc == 7))
                nc.vector.scalar_tensor_tensor(
                    qt_sb[:, pair, tsl], q_ps[:], bq_sb[:, pair:pair + 1],
                    wrep_sb[:], ALU.add, ALU.mult)
                # KT
                k_ps = ps_gen.tile([128, 512], F32, tag="gen")
                for dc in range(8):
                    nc.tensor.matmul(k_ps[:], wk_sb[:, dc, psl], xt_tg[:, dc, :],
                                     start=(dc == 0), stop=(dc == 7))
                nc.scalar.activation(kt_sb[:, pair, tsl], k_ps[:], AF.Identity,
                                     bias=bk_sb[:, pair:pair + 1])

            # V (+bias via rank-1 matmul)
            for tl in range(4):
                tc16 = tg * 4 + tl
                v_ps = ps_vn.tile([128, 256], F32, tag="v")
                for dc in range(8):
                    nc.tensor.matmul(v_ps[:], xt_tg[:, dc, tl * 128:(tl + 1) * 128],
                                     wv_sb[:, dc, :], start=(dc == 0), stop=False)
                nc.tensor.matmul(v_ps[:], ones_sb[:], bvr_sb[:],
                                 start=False, stop=True)
                if tl % 2 == 0:
                    nc.vector.tensor_copy(v_sb[:, tc16, :], v_ps[:])
                else:
                    nc.scalar.copy(v_sb[:, tc16, :], v_ps[:])

        # ================= stage 2+3: scores + ctx =================
        ndve = 0
        for pair in range(2 if 2 in phases else 0):
            for tg in range(NTG):
                tsl = slice(tg * 512, (tg + 1) * 512)
                ctx_ps = [ps_ctx.tile([64, 512], F32, tag="ctx", name=f"ctx{_h}") for _h in range(2)]
                nblk = 4 * tg + 4
                prev_sb, prev_j = None, -1
                for j in range(nblk):
                    st_list = []
                    for hh in range(2):
                        hsl = slice(64 * hh, 64 * hh + 64)
                        st_ps = ps_st.tile([128, 512], F32, tag="st")
                        nc.tensor.matmul(
                            st_ps[:], kt_sb[hsl, pair, j * 128:(j + 1) * 128],
                            qt_sb[hsl, pair, tsl], start=True, stop=True,
                            tile_position=(64 * hh, 0))
                        st_list.append(st_ps)
                    cur_sb = []
                    for hh in range(2):
                        st_sb = stp.tile([128, 512], BF16, tag="st_sb")
                        r = j - 4 * tg
                        if r >= 0:
                            nc.vector.tensor_mul(
                                st_sb[:], st_list[hh][:],
                                mask_sb[:, 384 - 128 * r: 896 - 128 * r])
                        else:
                            ndve += 1
                            if ndve % 4 == 0:
                                nc.vector.tensor_copy(st_sb[:], st_list[hh][:])
                            else:
                                nc.scalar.copy(st_sb[:], st_list[hh][:])
                        cur_sb.append(st_sb)
                    if prev_sb is not None:
                        for hh in range(2):
                            hl_g = 2 * pair + hh
                            nc.tensor.matmul(
                                ctx_ps[hh][:],
                                v_sb[:, prev_j, 64 * hl_g:64 * hl_g + 64],
                                prev_sb[hh][:],
                                start=(prev_j == 0), stop=False)
                    prev_sb, prev_j = cur_sb, j
                for hh in range(2):
                    hl_g = 2 * pair + hh
                    nc.tensor.matmul(
                        ctx_ps[hh][:],
                        v_sb[:, prev_j, 64 * hl_g:64 * hl_g + 64], prev_sb[hh][:],
                        start=(prev_j == 0), stop=True)
                for hh in range(2):
                    if (tg + hh) % 2 == 0:
                        nc.vector.tensor_copy(ctxt_sb[64*hh:64*hh+64, pair, tsl], ctx_ps[hh][:])
                    else:
                        nc.scalar.copy(ctxt_sb[64*hh:64*hh+64, pair, tsl], ctx_ps[hh][:])

        # ================= stage 4: out projection =================
        for tc16 in range(NTC if 3 in phases else 0):
            csl = slice(tc16 * 128, (tc16 + 1) * 128)
            out_sb = outp.tile([128, D], BF16, tag="out")
            for eb in range(2):
                esl = slice(eb * 512, (eb + 1) * 512)
                o_ps = ps_gen.tile([128, 512], F32, tag="gen")
                for pair in range(2):
                    nc.tensor.matmul(o_ps[:], ctxt_sb[:, pair, csl],
                                     wo_sb[:, pair, esl],
                                     start=(pair == 0), stop=(pair == 1))
                if eb == 0:
                    nc.vector.tensor_copy(out_sb[:, esl], o_ps[:])
                else:
                    nc.scalar.copy(out_sb[:, esl], o_ps[:])
            nc.sync.dma_start(out[csl, :], out_sb[:])


def build_nc(phases=(1, 2, 3), reps=1):
    nc = bacc.Bacc("TRN2", target_bir_lowering=False, debug=False,
                   enable_partition_id=False)
    blob = nc.dram_tensor("blob", [BLOB_N], BF16, kind="ExternalInput").ap()
    out = nc.dram_tensor("out", [S, D], BF16, kind="ExternalOutput").ap()
    with tile.TileContext(nc) as tc:
        _kernel_body(tc, out, blob, phases=phases, reps=reps)
    nc.compile()
    return nc


def _make_maskB():
    m = np.zeros((128, 896), dtype=np.float32)
    s = np.arange(128)[:, None]
    c = np.arange(896)[None, :]
    m[(c >= 384) & ((c - 384) >= s)] = 1.0
    m[:, 512:] = 1.0
    return m


def core_inputs(inp, c):
    b, hg = c // 4, c % 4
    heads = list(range(4 * hg, 4 * hg + 4))
    x = np.asarray(inp["x"], dtype=np.float32)
    Wqk = np.asarray(inp["Wqk"], dtype=np.float32)
    bqk = np.asarray(inp["bqk"], dtype=np.float32)
    Wv = np.asarray(inp["Wv"], dtype=np.float32)
    bv = np.asarray(inp["bv"], dtype=np.float32)
    Wn = np.asarray(inp["Wn"], dtype=np.float32)
    bn = np.asarray(inp["bn"], dtype=np.float32)
    Wo = np.asarray(inp["Wo"], dtype=np.float32)

    def perm_pac(w, a, c_):
        # [128*a, c_] -> flat in (p, a, c) order
        return np.ascontiguousarray(
            w.reshape(a, 128, c_).transpose(1, 0, 2)).reshape(-1)

    d = {}
    d["xt"] = x[b].T.reshape(-1)                       # (a p t) == d-major
    wq = np.concatenate([Wqk[:, h * 64:(h + 1) * 64] for h in heads], 1)
    wk = np.concatenate([Wqk[:, 1024 + h * 64:1024 + (h + 1) * 64] for h in heads], 1)
    wv = np.concatenate([Wv[:, h * 64:(h + 1) * 64] for h in heads], 1)
    wo = np.concatenate([Wo[h * 64:(h + 1) * 64, :] for h in heads], 0)
    d["wq"] = perm_pac(wq, 8, 256)
    d["wk"] = perm_pac(wk, 8, 256)
    d["wv"] = perm_pac(wv, 8, 256)
    d["wo"] = perm_pac(wo, 2, 1024)
    d["wn"] = perm_pac(Wn[:, heads], 8, HL)
    d["maskB"] = _make_maskB().reshape(-1)
    sel = np.zeros((4, 256), dtype=np.float32)
    for p in range(2):
        sel[2 * p + 0, 128 * p:128 * p + 64] = 1.0
        sel[2 * p + 1, 128 * p + 64:128 * p + 128] = 1.0
    d["sel"] = sel.reshape(-1)
    d["bq"] = np.concatenate([bqk[h * 64:(h + 1) * 64] for h in heads]).reshape(2, 128).T.reshape(-1)
    d["bk"] = np.concatenate([bqk[1024 + h * 64:1024 + (h + 1) * 64] for h in heads]).reshape(2, 128).T.reshape(-1)
    d["bvr"] = np.concatenate([bv[h * 64:(h + 1) * 64] for h in heads])
    d["ones"] = np.ones(128, dtype=np.float32)
    bnc = np.zeros(64, dtype=np.float32)
    bnc[:HL] = -bn[heads]
    d["bnc"] = bnc

    blob = np.zeros(BLOB_N, dtype=NPBF)
    for name, sz in _SECTS:
        v = d[name].astype(np.float32).reshape(-1)
        blob[_OFS[name]:_OFS[name] + v.size] = v.astype(NPBF)
    return {"blob": blob}


_NC_CACHE = {}


def _get_nc():
    if "nc" not in _NC_CACHE:
        _NC_CACHE["nc"] = build_nc()
    return _NC_CACHE["nc"]


def _run(inputs, **spmd_kwargs):
    nc = _get_nc()
    in_maps = [core_inputs(inputs, c) for c in range(8)]
    res = run_bass_kernel_spmd(nc, in_maps, list(range(8)), **spmd_kwargs)
    bo = np.asarray(inputs["bo"], dtype=np.float32)
    out = np.stack([
        res.results[0 + 4 * b]["out"].astype(np.float32)
        + res.results[1 + 4 * b]["out"].astype(np.float32)
        + res.results[2 + 4 * b]["out"].astype(np.float32)
        + res.results[3 + 4 * b]["out"].astype(np.float32) + bo
        for b in range(B)
    ])
    return out.astype(np.float32), res


def kernel(**inputs):
    out, _ = _run(inputs)
    return out


# revision 6
# speedup vs baseline: 118.3971x; 1.9391x over previous
"""MHNA (masked, exp(n)-normalized multi-head attention) Trainium2 Bass kernel.

Sharding: 8 cores = batch(2) x head-groups(4 heads each). Each core computes its
4 heads' attention + its slice of the output projection (Wo rows); host sums the
4 partial outputs per batch and adds bo.

I/O design (the dominant cost through this harness is per-call host<->device
I/O staging, not device compute): ALL inputs are packed into a single bf16
blob per core (one buffer instead of 17), and the partial output is written
in bf16. Device compute accumulates in fp32 PSUM throughout; only SBUF
residents are bf16.

Device layout (same structure as the validated fp32r version):
  - x is passed pre-transposed (xt = x[b].T) so every projection streams with
    the contraction dim (D) on partitions.
  - Q/K are produced transposed (QT/KT = W.T @ xT) in head-pair tiles [128, S]:
    partitions 0:64 = even head, 64:128 = odd head. Scores then run as K=64
    row-packed matmuls (two heads concurrently in the PE array).
  - scores*mask/exp(n_t): the normalizer is folded into Q (q_t scaled by
    exp(-n_t) before scores); the causal mask is a sliding-window tile
    maskB[128, 896] applied during PSUM->SBUF evacuation of diagonal blocks.
  - ctx is produced transposed (ctxT = V.T @ ST) with col-packed M=64 matmuls,
    feeding the out-projection directly as lhsT.
  - Biases: per-partition biases (bq/bk/bn) via ACT/DVE ops (converted to fp32
    tiles on device); the V bias (varies along the free dim) via a K=1 rank-1
    matmul accumulated into the same PSUM.
"""
import numpy as np

import concourse.bacc as bacc
import concourse.mybir as mybir
import concourse.tile as tile
from concourse.bass_utils import run_bass_kernel_spmd

F32 = mybir.dt.float32
BF16 = mybir.dt.bfloat16
NPBF = mybir.dt.np(mybir.dt.bfloat16)
AF = mybir.ActivationFunctionType
ALU = mybir.AluOpType

B, S, D, H, DH = 2, 2048, 1024, 16, 64
HL = 4            # heads per core
NTG = 4           # t groups of 512
NTC = 16          # t chunks of 128

# Packed-blob layout: (name, n_elems). Offsets accumulate in order; each
# section is 64-element aligned. Element order per section is chosen so the
# SBUF-load DMA sees large contiguous per-partition chunks:
#   xt   (a p t)  : xt[d=a*128+p, t] natural order, d-major
#   wq/wk/wv (p a c), wo (p a e), wn (p a c): per-partition contiguous
#   maskB/bq/bk (p c), sel (h c), bvr/ones (1 c), bnc (h 1)
_SECTS = [
    ("xt", D * S), ("wq", D * 256), ("wk", D * 256), ("wv", D * 256),
    ("wo", 256 * D), ("wn", D * HL), ("maskB", 128 * 896), ("sel", HL * 256),
    ("bq", 128 * 2), ("bk", 128 * 2), ("bvr", 256), ("ones", 128), ("bnc", 64),
]
_OFS = {}
_cur = 0
for _n, _sz in _SECTS:
    _OFS[_n] = _cur
    _cur += (_sz + 63) // 64 * 64
BLOB_N = _cur


def _views(blob):
    o = _OFS
    def sl(name, n):
        return blob[o[name]:o[name] + n]
    return dict(
        xt=sl("xt", D * S).rearrange("(a p t) -> p a t", p=128, t=S),
        wq=sl("wq", D * 256).rearrange("(p a c) -> p a c", a=8, c=256),
        wk=sl("wk", D * 256).rearrange("(p a c) -> p a c", a=8, c=256),
        wv=sl("wv", D * 256).rearrange("(p a c) -> p a c", a=8, c=256),
        wo=sl("wo", 256 * D).rearrange("(p a e) -> p a e", a=2, e=D),
        wn=sl("wn", D * HL).rearrange("(p a c) -> p a c", a=8, c=HL),
        maskB=sl("maskB", 128 * 896).rearrange("(p c) -> p c", c=896),
        sel=sl("sel", HL * 256).rearrange("(h c) -> h c", c=256),
        bq=sl("bq", 128 * 2).rearrange("(p c) -> p c", c=2),
        bk=sl("bk", 128 * 2).rearrange("(p c) -> p c", c=2),
        bvr=sl("bvr", 256).rearrange("(o c) -> o c", o=1),
        ones=sl("ones", 128).rearrange("(o c) -> o c", o=1),
        bnc=sl("bnc", HL).rearrange("(h o) -> h o", o=1),
    )


def _kernel_body(tc, out, blob, phases=(1, 2, 3), reps=1, hw_loop=1):
    nc = tc.nc
    ins = _views(blob)
    with (
        tc.tile_pool(name="const", bufs=1) as cp,
        tc.tile_pool(name="xtp", bufs=2) as xtp,
        tc.tile_pool(name="big", bufs=2) as bigp,
        tc.tile_pool(name="stp", bufs=6) as stp,
        tc.tile_pool(name="outp", bufs=2) as outp,
        tc.tile_pool(name="ps_st", bufs=3, space="PSUM") as ps_st,
        tc.tile_pool(name="ps_ctx", bufs=2, space="PSUM") as ps_ctx,
        tc.tile_pool(name="ps_gen", bufs=2, space="PSUM") as ps_gen,
        tc.tile_pool(name="ps_vn", bufs=1, space="PSUM") as ps_vn,
    ):
        def emit():
            for _rep in range(reps):
                _one_pass(nc, out, ins, phases, cp, xtp, bigp, stp, outp,
                          ps_st, ps_ctx, ps_gen, ps_vn)
        if hw_loop > 1:
            with tc.For_i(0, hw_loop):
                emit()
        else:
            emit()


def _one_pass(nc, out, ins, phases, cp, xtp, bigp, stp, outp,
              ps_st, ps_ctx, ps_gen, ps_vn):
        # ---- constants / weights to SBUF ----
        wq_sb = cp.tile([128, 8, 256], BF16)
        wk_sb = cp.tile([128, 8, 256], BF16)
        wv_sb = cp.tile([128, 8, 256], BF16)
        wn_sb = cp.tile([128, 8, HL], BF16)
        wo_sb = cp.tile([128, 2, D], BF16)
        for name, t in (("wq", wq_sb), ("wk", wk_sb), ("wv", wv_sb),
                        ("wn", wn_sb), ("wo", wo_sb)):
            nc.sync.dma_start(t[:], ins[name])
        bq_bf = cp.tile([128, 2], BF16)
        bk_bf = cp.tile([128, 2], BF16)
        bnc_bf = cp.tile([HL, 1], BF16)
        bvr_sb = cp.tile([1, 256], BF16)
        sel_sb = cp.tile([HL, 256], BF16)
        mask_sb = cp.tile([128, 896], BF16)
        ones_sb = cp.tile([1, 128], BF16)
        for name, t in (("bq", bq_bf), ("bk", bk_bf), ("bvr", bvr_sb),
                        ("bnc", bnc_bf), ("sel", sel_sb), ("maskB", mask_sb),
                        ("ones", ones_sb)):
            nc.sync.dma_start(t[:], ins[name])
        # per-partition bias operands need fp32
        bq_sb = cp.tile([128, 2], F32)
        bk_sb = cp.tile([128, 2], F32)
        bnc_sb = cp.tile([HL, 1], F32)
        nc.scalar.copy(bq_sb[:], bq_bf[:])
        nc.scalar.copy(bk_sb[:], bk_bf[:])
        nc.scalar.copy(bnc_sb[:], bnc_bf[:])

        qt_sb = bigp.tile([128, 2, S], BF16)      # [part, pair, t]
        kt_sb = bigp.tile([128, 2, S], BF16)
        v_sb = bigp.tile([128, NTC, 256], BF16)   # [s-in-chunk, chunk, hc]
        wt_sb = bigp.tile([HL, S], BF16)          # exp(-(n+bn)) per local head
        ctxt_sb = bigp.tile([128, 2, S], BF16)    # [pair-dv, pair, t]

        # ================= stage 1: projections =================
        for tg in range(NTG if 1 in phases else 0):
            tsl = slice(tg * 512, (tg + 1) * 512)
            xt_tg = xtp.tile([128, 8, 512], BF16, tag="xt")
            nc.sync.dma_start(xt_tg[:], ins["xt"][:, :, tsl])

            # N-projection -> wT = exp(-(n_pre + bn))
            n_ps = ps_vn.tile([HL, 512], F32, tag="v")
            for dc in range(8):
                nc.tensor.matmul(n_ps[:], wn_sb[:, dc, :], xt_tg[:, dc, :],
                                 start=(dc == 0), stop=(dc == 7))
            nc.scalar.activation(wt_sb[:, tsl], n_ps[:], AF.Exp,
                                 bias=bnc_sb[:], scale=-1.0)

            for pair in range(2):
                psl = slice(128 * pair, 128 * pair + 128)
                # wrep[p, t] = exp(-n) broadcast: partitions 0:64 <- even head
                wrep_ps = ps_gen.tile([128, 512], F32, tag="gen")
                nc.tensor.matmul(wrep_ps[:], sel_sb[:, psl], wt_sb[:, tsl],
                                 start=True, stop=True)
                wrep_sb = outp.tile([128, 512], F32, tag="wrep_sb")
                nc.scalar.copy(wrep_sb[:], wrep_ps[:])
                # QT
                q_ps = ps_gen.tile([128, 512], F32, tag="gen")
                for dc in range(8):
                    nc.tensor.matmul(q_ps[:], wq_sb[:, dc, psl], xt_tg[:, dc, :],
                                     start=(dc == 0), stop=(dc == 7))
                nc.vector.scalar_tensor_tensor(
                    qt_sb[:, pair, tsl], q_ps[:], bq_sb[:, pair:pair + 1],
                    wrep_sb[:], ALU.add, ALU.mult)
                # KT
                k_ps = ps_gen.tile([128, 512], F32, tag="gen")
                for dc in range(8):
                    nc.tensor.matmul(k_ps[:], wk_sb[:, dc, psl], xt_tg[:, dc, :],
                                     start=(dc == 0), stop=(dc == 7))
                nc.scalar.activation(kt_sb[:, pair, tsl], k_ps[:], AF.Identity,
                                     bias=bk_sb[:, pair:pair + 1])

            # V (+bias via rank-1 matmul)
            for tl in range(4):
                tc16 = tg * 4 + tl
                v_ps = ps_vn.tile([128, 256], F32, tag="v")
                for dc in range(8):
                    nc.tensor.matmul(v_ps[:], xt_tg[:, dc, tl * 128:(tl + 1) * 128],
                                     wv_sb[:, dc, :], start=(dc == 0), stop=False)
                nc.tensor.matmul(v_ps[:], ones_sb[:], bvr_sb[:],
                                 start=False, stop=True)
                if tl % 2 == 0:
                    nc.vector.tensor_copy(v_sb[:, tc16, :], v_ps[:])
                else:
                    nc.scalar.copy(v_sb[:, tc16, :], v_ps[:])

        # ================= stage 2+3: scores + ctx =================
        ndve = 0
        for pair in range(2 if 2 in phases else 0):
            for tg in range(NTG):
                tsl = slice(tg * 512, (tg + 1) * 512)
                ctx_ps = [ps_ctx.tile([64, 512], F32, tag="ctx", name=f"ctx{_h}") for _h in range(2)]
                nblk = 4 * tg + 4
                prev_sb, prev_j = None, -1
                for j in range(nblk):
                    st_list = []
                    for hh in range(2):
                        hsl = slice(64 * hh, 64 * hh + 64)
                        st_ps = ps_st.tile([128, 512], F32, tag="st")
                        nc.tensor.matmul(
                            st_ps[:], kt_sb[hsl, pair, j * 128:(j + 1) * 128],
                            qt_sb[hsl, pair, tsl], start=True, stop=True,
                            tile_position=(64 * hh, 0))
                        st_list.append(st_ps)
                    cur_sb = []
                    for hh in range(2):
                        st_sb = stp.tile([128, 512], BF16, tag="st_sb")
                        r = j - 4 * tg
                        if r >= 0:
                            nc.vector.tensor_mul(
                                st_sb[:], st_list[hh][:],
                                mask_sb[:, 384 - 128 * r: 896 - 128 * r])
                        else:
                            ndve += 1
                            if ndve % 4 == 0:
                                nc.vector.tensor_copy(st_sb[:], st_list[hh][:])
                            else:
                                nc.scalar.copy(st_sb[:], st_list[hh][:])
                        cur_sb.append(st_sb)
                    if prev_sb is not None:
                        for hh in range(2):
                            hl_g = 2 * pair + hh
                            nc.tensor.matmul(
                                ctx_ps[hh][:],
                                v_sb[:, prev_j, 64 * hl_g:64 * hl_g + 64],
                                prev_sb[hh][:],
                                start=(prev_j == 0), stop=False)
                    prev_sb, prev_j = cur_sb, j
                for hh in range(2):
                    hl_g = 2 * pair + hh
                    nc.tensor.matmul(
                        ctx_ps[hh][:],
                        v_sb[:, prev_j, 64 * hl_g:64 * hl_g + 64], prev_sb[hh][:],
                        start=(prev_j == 0), stop=True)
                for hh in range(2):
                    if (tg + hh) % 2 == 0:
                        nc.vector.tensor_copy(ctxt_sb[64*hh:64*hh+64, pair, tsl], ctx_ps[hh][:])
                    else:
                        nc.scalar.copy(ctxt_sb[64*hh:64*hh+64, pair, tsl], ctx_ps[hh][:])

        # ================= stage 4: out projection =================
        for tc16 in range(NTC if 3 in phases else 0):
            csl = slice(tc16 * 128, (tc16 + 1) * 128)
            out_sb = outp.tile([128, D], BF16, tag="out")
            for eb in range(2):
                esl = slice(eb * 512, (eb + 1) * 512)
                o_ps = ps_gen.tile([128, 512], F32, tag="gen")
                for pair in range(2):
                    nc.tensor.matmul(o_ps[:], ctxt_sb[:, pair, csl],
                                     wo_sb[:, pair, esl],
                                     start=(pair == 0), stop=(pair == 1))
                if eb == 0:
                    nc.vector.tensor_copy(out_sb[:, esl], o_ps[:])
                else:
                    nc.scalar.copy(out_sb[:, esl], o_ps[:])
            nc.sync.dma_start(out[csl, :], out_sb[:])


def build_nc(phases=(1, 2, 3), reps=1, hw_loop=1):
    nc = bacc.Bacc("TRN2", target_bir_lowering=False, debug=False,
                   enable_partition_id=False)
    blob = nc.dram_tensor("blob", [BLOB_N], BF16, kind="ExternalInput").ap()
    out = nc.dram_tensor("out", [S, D], BF16, kind="ExternalOutput").ap()
    with tile.TileContext(nc) as tc:
        _kernel_body(tc, out, blob, phases=phases, reps=reps, hw_loop=hw_loop)
    nc.compile()
    return nc


def _make_maskB():
    m = np.zeros((128, 896), dtype=np.float32)
    s = np.arange(128)[:, None]
    c = np.arange(896)[None, :]
    m[(c >= 384) & ((c - 384) >= s)] = 1.0
    m[:, 512:] = 1.0
    return m


def core_inputs(inp, c):
    b, hg = c // 4, c % 4
    heads = list(range(4 * hg, 4 * hg + 4))
    x = np.asarray(inp["x"], dtype=np.float32)
    Wqk = np.asarray(inp["Wqk"], dtype=np.float32)
    bqk = np.asarray(inp["bqk"], dtype=np.float32)
    Wv = np.asarray(inp["Wv"], dtype=np.float32)
    bv = np.asarray(inp["bv"], dtype=np.float32)
    Wn = np.asarray(inp["Wn"], dtype=np.float32)
    bn = np.asarray(inp["bn"], dtype=np.float32)
    Wo = np.asarray(inp["Wo"], dtype=np.float32)

    def perm_pac(w, a, c_):
        # [128*a, c_] -> flat in (p, a, c) order
        return np.ascontiguousarray(
            w.reshape(a, 128, c_).transpose(1, 0, 2)).reshape(-1)

    d = {}
    d["xt"] = x[b].T.reshape(-1)                       # (a p t) == d-major
    wq = np.concatenate([Wqk[:, h * 64:(h + 1) * 64] for h in heads], 1)
    wk = np.concatenate([Wqk[:, 1024 + h * 64:1024 + (h + 1) * 64] for h in heads], 1)
    wv = np.concatenate([Wv[:, h * 64:(h + 1) * 64] for h in heads], 1)
    wo = np.concatenate([Wo[h * 64:(h + 1) * 64, :] for h in heads], 0)
    d["wq"] = perm_pac(wq, 8, 256)
    d["wk"] = perm_pac(wk, 8, 256)
    d["wv"] = perm_pac(wv, 8, 256)
    d["wo"] = perm_pac(wo, 2, 1024)
    d["wn"] = perm_pac(Wn[:, heads], 8, HL)
    d["maskB"] = _make_maskB().reshape(-1)
    sel = np.zeros((4, 256), dtype=np.float32)
    for p in range(2):
        sel[2 * p + 0, 128 * p:128 * p + 64] = 1.0
        sel[2 * p + 1, 128 * p + 64:128 * p + 128] = 1.0
    d["sel"] = sel.reshape(-1)
    d["bq"] = np.concatenate([bqk[h * 64:(h + 1) * 64] for h in heads]).reshape(2, 128).T.reshape(-1)
    d["bk"] = np.concatenate([bqk[1024 + h * 64:1024 + (h + 1) * 64] for h in heads]).reshape(2, 128).T.reshape(-1)
    d["bvr"] = np.concatenate([bv[h * 64:(h + 1) * 64] for h in heads])
    d["ones"] = np.ones(128, dtype=np.float32)
    bnc = np.zeros(64, dtype=np.float32)
    bnc[:HL] = -bn[heads]
    d["bnc"] = bnc

    blob = np.zeros(BLOB_N, dtype=NPBF)
    for name, sz in _SECTS:
        v = d[name].astype(np.float32).reshape(-1)
        blob[_OFS[name]:_OFS[name] + v.size] = v.astype(NPBF)
    return {"blob": blob}


_NC_CACHE = {}


def _get_nc():
    if "nc" not in _NC_CACHE:
        _NC_CACHE["nc"] = build_nc()
    return _NC_CACHE["nc"]


def _run(inputs, **spmd_kwargs):
    nc = _get_nc()
    in_maps = [core_inputs(inputs, c) for c in range(8)]
    res = run_bass_kernel_spmd(nc, in_maps, list(range(8)), **spmd_kwargs)
    bo = np.asarray(inputs["bo"], dtype=np.float32)
    out = np.stack([
        res.results[0 + 4 * b]["out"].astype(np.float32)
        + res.results[1 + 4 * b]["out"].astype(np.float32)
        + res.results[2 + 4 * b]["out"].astype(np.float32)
        + res.results[3 + 4 * b]["out"].astype(np.float32) + bo
        for b in range(B)
    ])
    return out.astype(np.float32), res


def kernel(**inputs):
    out, _ = _run(inputs)
    return out


# revision 7
# speedup vs baseline: 140.5720x; 1.1873x over previous
"""MHNA (masked, exp(n)-normalized multi-head attention) Trainium2 Bass kernel.

Sharding: 8 cores = batch(2) x head-groups(4 heads each). Each core computes its
4 heads' attention + its slice of the output projection (Wo rows); host sums the
4 partial outputs per batch and adds bo.

I/O design (the dominant cost through this harness is per-call host<->device
I/O staging, not device compute): ALL inputs are packed into a single bf16
blob per core (one buffer instead of 17), and the partial output is written
in bf16. Device compute accumulates in fp32 PSUM throughout; only SBUF
residents are bf16.

Device layout (same structure as the validated fp32r version):
  - x is passed pre-transposed (xt = x[b].T) so every projection streams with
    the contraction dim (D) on partitions.
  - Q/K are produced transposed (QT/KT = W.T @ xT) in head-pair tiles [128, S]:
    partitions 0:64 = even head, 64:128 = odd head. Scores then run as K=64
    row-packed matmuls (two heads concurrently in the PE array).
  - scores*mask/exp(n_t): the normalizer is folded into Q (q_t scaled by
    exp(-n_t) before scores); the causal mask is a sliding-window tile
    maskB[128, 896] applied during PSUM->SBUF evacuation of diagonal blocks.
  - ctx is produced transposed (ctxT = V.T @ ST) with col-packed M=64 matmuls,
    feeding the out-projection directly as lhsT.
  - Biases: per-partition biases (bq/bk/bn) via ACT/DVE ops (converted to fp32
    tiles on device); the V bias (varies along the free dim) via a K=1 rank-1
    matmul accumulated into the same PSUM.
"""
import numpy as np

import concourse.bacc as bacc
import concourse.mybir as mybir
import concourse.tile as tile
from concourse.bass_utils import run_bass_kernel_spmd

F32 = mybir.dt.float32
BF16 = mybir.dt.bfloat16
NPBF = mybir.dt.np(mybir.dt.bfloat16)
AF = mybir.ActivationFunctionType
ALU = mybir.AluOpType

B, S, D, H, DH = 2, 2048, 1024, 16, 64
HL = 4            # heads per core
NTG = 4           # t groups of 512
NTC = 16          # t chunks of 128

# Packed-blob layout: (name, n_elems). Offsets accumulate in order; each
# section is 64-element aligned. Element order per section is chosen so the
# SBUF-load DMA sees large contiguous per-partition chunks:
#   xt   (a p t)  : xt[d=a*128+p, t] natural order, d-major
#   wq/wk/wv (p a c), wo (p a e), wn (p a c): per-partition contiguous
#   maskB/bq/bk (p c), sel (h c), bvr/ones (1 c), bnc (h 1)
_SECTS = [
    ("xt", D * S), ("wq", D * 256), ("wk", D * 256), ("wv", D * 256),
    ("wo", 256 * D), ("wn", D * HL), ("maskB", 128 * 896), ("sel", HL * 256),
    ("bq", 128 * 2), ("bk", 128 * 2), ("bvr", 256), ("ones", 128), ("bnc", 64),
]
_OFS = {}
_cur = 0
for _n, _sz in _SECTS:
    _OFS[_n] = _cur
    _cur += (_sz + 63) // 64 * 64
BLOB_N = _cur


def _views(blob):
    o = _OFS
    def sl(name, n):
        return blob[o[name]:o[name] + n]
    return dict(
        xt=sl("xt", D * S).rearrange("(a p t) -> p a t", p=128, t=S),
        wq=sl("wq", D * 256).rearrange("(p a c) -> p a c", a=8, c=256),
        wk=sl("wk", D * 256).rearrange("(p a c) -> p a c", a=8, c=256),
        wv=sl("wv", D * 256).rearrange("(p a c) -> p a c", a=8, c=256),
        wo=sl("wo", 256 * D).rearrange("(p a e) -> p a e", a=2, e=D),
        wn=sl("wn", D * HL).rearrange("(p a c) -> p a c", a=8, c=HL),
        maskB=sl("maskB", 128 * 896).rearrange("(p c) -> p c", c=896),
        sel=sl("sel", HL * 256).rearrange("(h c) -> h c", c=256),
        bq=sl("bq", 128 * 2).rearrange("(p c) -> p c", c=2),
        bk=sl("bk", 128 * 2).rearrange("(p c) -> p c", c=2),
        bvr=sl("bvr", 256).rearrange("(o c) -> o c", o=1),
        ones=sl("ones", 128).rearrange("(o c) -> o c", o=1),
        bnc=sl("bnc", HL).rearrange("(h o) -> h o", o=1),
    )


def _kernel_body(tc, out, blob, phases=(1, 2, 3), reps=1, hw_loop=1):
    nc = tc.nc
    ins = _views(blob)
    with (
        tc.tile_pool(name="const", bufs=1) as cp,
        tc.tile_pool(name="xtp", bufs=2) as xtp,
        tc.tile_pool(name="big", bufs=2) as bigp,
        tc.tile_pool(name="stp", bufs=6) as stp,
        tc.tile_pool(name="outp", bufs=2) as outp,
        tc.tile_pool(name="ps_st", bufs=3, space="PSUM") as ps_st,
        tc.tile_pool(name="ps_ctx", bufs=2, space="PSUM") as ps_ctx,
        tc.tile_pool(name="ps_gen", bufs=2, space="PSUM") as ps_gen,
        tc.tile_pool(name="ps_vn", bufs=1, space="PSUM") as ps_vn,
    ):
        def emit():
            for _rep in range(reps):
                _one_pass(nc, out, ins, phases, cp, xtp, bigp, stp, outp,
                          ps_st, ps_ctx, ps_gen, ps_vn)
        if hw_loop > 1:
            with tc.For_i(0, hw_loop):
                emit()
        else:
            emit()


def _one_pass(nc, out, ins, phases, cp, xtp, bigp, stp, outp,
              ps_st, ps_ctx, ps_gen, ps_vn):
        # ---- constants / weights to SBUF ----
        wq_sb = cp.tile([128, 8, 256], BF16)
        wk_sb = cp.tile([128, 8, 256], BF16)
        wv_sb = cp.tile([128, 8, 256], BF16)
        wn_sb = cp.tile([128, 8, HL], BF16)
        wo_sb = cp.tile([128, 2, D], BF16)
        for name, t in (("wq", wq_sb), ("wk", wk_sb), ("wv", wv_sb),
                        ("wn", wn_sb), ("wo", wo_sb)):
            nc.sync.dma_start(t[:], ins[name])
        bq_bf = cp.tile([128, 2], BF16)
        bk_bf = cp.tile([128, 2], BF16)
        bnc_bf = cp.tile([HL, 1], BF16)
        bvr_sb = cp.tile([1, 256], BF16)
        sel_sb = cp.tile([HL, 256], BF16)
        mask_sb = cp.tile([128, 896], BF16)
        ones_sb = cp.tile([1, 128], BF16)
        for name, t in (("bq", bq_bf), ("bk", bk_bf), ("bvr", bvr_sb),
                        ("bnc", bnc_bf), ("sel", sel_sb), ("maskB", mask_sb),
                        ("ones", ones_sb)):
            nc.sync.dma_start(t[:], ins[name])
        # per-partition bias operands need fp32
        bq_sb = cp.tile([128, 2], F32)
        bk_sb = cp.tile([128, 2], F32)
        bnc_sb = cp.tile([HL, 1], F32)
        nc.scalar.copy(bq_sb[:], bq_bf[:])
        nc.scalar.copy(bk_sb[:], bk_bf[:])
        nc.scalar.copy(bnc_sb[:], bnc_bf[:])

        qt_sb = bigp.tile([128, 2, S], BF16)      # [part, pair, t]
        kt_sb = bigp.tile([128, 2, S], BF16)
        v_sb = bigp.tile([128, NTC, 256], BF16)   # [s-in-chunk, chunk, hc]
        wt_sb = bigp.tile([HL, S], BF16)          # exp(-(n+bn)) per local head
        ctxt_sb = bigp.tile([128, 2, S], BF16)    # [pair-dv, pair, t]

        # ================= stage 1: projections =================
        for tg in range(NTG if 1 in phases else 0):
            tsl = slice(tg * 512, (tg + 1) * 512)
            xt_tg = xtp.tile([128, 8, 512], BF16, tag="xt")
            nc.sync.dma_start(xt_tg[:], ins["xt"][:, :, tsl])

            # N-projection -> wT = exp(-(n_pre + bn))
            n_ps = ps_vn.tile([HL, 512], F32, tag="v")
            for dc in range(8):
                nc.tensor.matmul(n_ps[:], wn_sb[:, dc, :], xt_tg[:, dc, :],
                                 start=(dc == 0), stop=(dc == 7))
            nc.scalar.activation(wt_sb[:, tsl], n_ps[:], AF.Exp,
                                 bias=bnc_sb[:], scale=-1.0)

            for pair in range(2):
                psl = slice(128 * pair, 128 * pair + 128)
                # wrep[p, t] = exp(-n) broadcast: partitions 0:64 <- even head
                wrep_ps = ps_gen.tile([128, 512], F32, tag="gen")
                nc.tensor.matmul(wrep_ps[:], sel_sb[:, psl], wt_sb[:, tsl],
                                 start=True, stop=True)
                wrep_sb = outp.tile([128, 512], F32, tag="wrep_sb")
                nc.scalar.copy(wrep_sb[:], wrep_ps[:])
                # QT
                q_ps = ps_gen.tile([128, 512], F32, tag="gen")
                for dc in range(8):
                    nc.tensor.matmul(q_ps[:], wq_sb[:, dc, psl], xt_tg[:, dc, :],
                                     start=(dc == 0), stop=(dc == 7))
                nc.vector.scalar_tensor_tensor(
                    qt_sb[:, pair, tsl], q_ps[:], bq_sb[:, pair:pair + 1],
                    wrep_sb[:], ALU.add, ALU.mult)
                # KT
                k_ps = ps_gen.tile([128, 512], F32, tag="gen")
                for dc in range(8):
                    nc.tensor.matmul(k_ps[:], wk_sb[:, dc, psl], xt_tg[:, dc, :],
                                     start=(dc == 0), stop=(dc == 7))
                nc.scalar.activation(kt_sb[:, pair, tsl], k_ps[:], AF.Identity,
                                     bias=bk_sb[:, pair:pair + 1])

            # V (+bias via rank-1 matmul)
            for tl in range(4):
                tc16 = tg * 4 + tl
                v_ps = ps_vn.tile([128, 256], F32, tag="v")
                for dc in range(8):
                    nc.tensor.matmul(v_ps[:], xt_tg[:, dc, tl * 128:(tl + 1) * 128],
                                     wv_sb[:, dc, :], start=(dc == 0), stop=False)
                nc.tensor.matmul(v_ps[:], ones_sb[:], bvr_sb[:],
                                 start=False, stop=True)
                if tl % 2 == 0:
                    nc.vector.tensor_copy(v_sb[:, tc16, :], v_ps[:])
                else:
                    nc.scalar.copy(v_sb[:, tc16, :], v_ps[:])

        # ================= stage 2+3: scores + ctx =================
        ndve = 0
        for pair in range(2 if 2 in phases else 0):
            for tg in range(NTG):
                tsl = slice(tg * 512, (tg + 1) * 512)
                ctx_ps = [ps_ctx.tile([64, 512], F32, tag="ctx", name=f"ctx{_h}") for _h in range(2)]
                nblk = 4 * tg + 4
                prev_sb, prev_j, prev_q0 = None, -1, 0
                for j in range(nblk):
                    r = j - 4 * tg
                    # causal: keys block j only affects queries >= 128*j, i.e.
                    # local query columns >= 128*r. Skip the always-masked part.
                    q0 = 128 * r if r > 0 else 0
                    st_list = []
                    for hh in range(2):
                        hsl = slice(64 * hh, 64 * hh + 64)
                        st_ps = ps_st.tile([128, 512], F32, tag="st")
                        nc.tensor.matmul(
                            st_ps[:, q0:], kt_sb[hsl, pair, j * 128:(j + 1) * 128],
                            qt_sb[hsl, pair, tg * 512 + q0:(tg + 1) * 512],
                            start=True, stop=True,
                            tile_position=(64 * hh, 0))
                        st_list.append(st_ps)
                    cur_sb = []
                    for hh in range(2):
                        st_sb = stp.tile([128, 512], BF16, tag="st_sb")
                        if r >= 0:
                            nc.vector.tensor_mul(
                                st_sb[:, q0:], st_list[hh][:, q0:],
                                mask_sb[:, 384: 896 - q0])
                        else:
                            ndve += 1
                            if ndve % 4 == 0:
                                nc.vector.tensor_copy(st_sb[:], st_list[hh][:])
                            else:
                                nc.scalar.copy(st_sb[:], st_list[hh][:])
                        cur_sb.append(st_sb)
                    if prev_sb is not None:
                        for hh in range(2):
                            hl_g = 2 * pair + hh
                            nc.tensor.matmul(
                                ctx_ps[hh][:, prev_q0:],
                                v_sb[:, prev_j, 64 * hl_g:64 * hl_g + 64],
                                prev_sb[hh][:, prev_q0:],
                                start=(prev_j == 0), stop=False)
                    prev_sb, prev_j, prev_q0 = cur_sb, j, q0
                for hh in range(2):
                    hl_g = 2 * pair + hh
                    nc.tensor.matmul(
                        ctx_ps[hh][:, prev_q0:],
                        v_sb[:, prev_j, 64 * hl_g:64 * hl_g + 64],
                        prev_sb[hh][:, prev_q0:],
                        start=(prev_j == 0), stop=True)
                for hh in range(2):
                    if (tg + hh) % 2 == 0:
                        nc.vector.tensor_copy(ctxt_sb[64*hh:64*hh+64, pair, tsl], ctx_ps[hh][:])
                    else:
                        nc.scalar.copy(ctxt_sb[64*hh:64*hh+64, pair, tsl], ctx_ps[hh][:])

        # ================= stage 4: out projection =================
        for tc16 in range(NTC if 3 in phases else 0):
            csl = slice(tc16 * 128, (tc16 + 1) * 128)
            out_sb = outp.tile([128, D], BF16, tag="out")
            for eb in range(2):
                esl = slice(eb * 512, (eb + 1) * 512)
                o_ps = ps_gen.tile([128, 512], F32, tag="gen")
                for pair in range(2):
                    nc.tensor.matmul(o_ps[:], ctxt_sb[:, pair, csl],
                                     wo_sb[:, pair, esl],
                                     start=(pair == 0), stop=(pair == 1))
                if eb == 0:
                    nc.vector.tensor_copy(out_sb[:, esl], o_ps[:])
                else:
                    nc.scalar.copy(out_sb[:, esl], o_ps[:])
            nc.sync.dma_start(out[csl, :], out_sb[:])


def build_nc(phases=(1, 2, 3), reps=1, hw_loop=1):
    nc = bacc.Bacc("TRN2", target_bir_lowering=False, debug=False,
                   enable_partition_id=False)
    blob = nc.dram_tensor("blob", [BLOB_N], BF16, kind="ExternalInput").ap()
    out = nc.dram_tensor("out", [S, D], BF16, kind="ExternalOutput").ap()
    with tile.TileContext(nc) as tc:
        _kernel_body(tc, out, blob, phases=phases, reps=reps, hw_loop=hw_loop)
    nc.compile()
    return nc


def _make_maskB():
    m = np.zeros((128, 896), dtype=np.float32)
    s = np.arange(128)[:, None]
    c = np.arange(896)[None, :]
    m[(c >= 384) & ((c - 384) >= s)] = 1.0
    m[:, 512:] = 1.0
    return m


def core_inputs(inp, c):
    b, hg = c // 4, c % 4
    heads = list(range(4 * hg, 4 * hg + 4))
    x = np.asarray(inp["x"], dtype=np.float32)
    Wqk = np.asarray(inp["Wqk"], dtype=np.float32)
    bqk = np.asarray(inp["bqk"], dtype=np.float32)
    Wv = np.asarray(inp["Wv"], dtype=np.float32)
    bv = np.asarray(inp["bv"], dtype=np.float32)
    Wn = np.asarray(inp["Wn"], dtype=np.float32)
    bn = np.asarray(inp["bn"], dtype=np.float32)
    Wo = np.asarray(inp["Wo"], dtype=np.float32)

    def perm_pac(w, a, c_):
        # [128*a, c_] -> flat in (p, a, c) order
        return np.ascontiguousarray(
            w.reshape(a, 128, c_).transpose(1, 0, 2)).reshape(-1)

    d = {}
    d["xt"] = x[b].T.reshape(-1)                       # (a p t) == d-major
    wq = np.concatenate([Wqk[:, h * 64:(h + 1) * 64] for h in heads], 1)
    wk = np.concatenate([Wqk[:, 1024 + h * 64:1024 + (h + 1) * 64] for h in heads], 1)
    wv = np.concatenate([Wv[:, h * 64:(h + 1) * 64] for h in heads], 1)
    wo = np.concatenate([Wo[h * 64:(h + 1) * 64, :] for h in heads], 0)
    d["wq"] = perm_pac(wq, 8, 256)
    d["wk"] = perm_pac(wk, 8, 256)
    d["wv"] = perm_pac(wv, 8, 256)
    d["wo"] = perm_pac(wo, 2, 1024)
    d["wn"] = perm_pac(Wn[:, heads], 8, HL)
    d["maskB"] = _make_maskB().reshape(-1)
    sel = np.zeros((4, 256), dtype=np.float32)
    for p in range(2):
        sel[2 * p + 0, 128 * p:128 * p + 64] = 1.0
        sel[2 * p + 1, 128 * p + 64:128 * p + 128] = 1.0
    d["sel"] = sel.reshape(-1)
    d["bq"] = np.concatenate([bqk[h * 64:(h + 1) * 64] for h in heads]).reshape(2, 128).T.reshape(-1)
    d["bk"] = np.concatenate([bqk[1024 + h * 64:1024 + (h + 1) * 64] for h in heads]).reshape(2, 128).T.reshape(-1)
    d["bvr"] = np.concatenate([bv[h * 64:(h + 1) * 64] for h in heads])
    d["ones"] = np.ones(128, dtype=np.float32)
    bnc = np.zeros(64, dtype=np.float32)
    bnc[:HL] = -bn[heads]
    d["bnc"] = bnc

    blob = np.zeros(BLOB_N, dtype=NPBF)
    for name, sz in _SECTS:
        v = d[name].astype(np.float32).reshape(-1)
        blob[_OFS[name]:_OFS[name] + v.size] = v.astype(NPBF)
    return {"blob": blob}


_NC_CACHE = {}


def _get_nc():
    if "nc" not in _NC_CACHE:
        _NC_CACHE["nc"] = build_nc()
    return _NC_CACHE["nc"]


def _run(inputs, **spmd_kwargs):
    nc = _get_nc()
    in_maps = [core_inputs(inputs, c) for c in range(8)]
    res = run_bass_kernel_spmd(nc, in_maps, list(range(8)), **spmd_kwargs)
    bo = np.asarray(inputs["bo"], dtype=np.float32)
    out = np.stack([
        res.results[0 + 4 * b]["out"].astype(np.float32)
        + res.results[1 + 4 * b]["out"].astype(np.float32)
        + res.results[2 + 4 * b]["out"].astype(np.float32)
        + res.results[3 + 4 * b]["out"].astype(np.float32) + bo
        for b in range(B)
    ])
    return out.astype(np.float32), res


def kernel(**inputs):
    out, _ = _run(inputs)
    return out
